# revision 9
# baseline (speedup 1.0000x reference)
"""LongTermAttention (continuous softmax over Gaussian RBF basis) — Trainium2 Bass kernel.

Sharding: 8 cores, tensor-parallel over heads (2 heads/core).

Math restructuring (vs the reference):
  * mu/sigma are linear functionals of q — the [1,H,Q,N] score tensor is never
    materialized.  The whole mu/sigma path collapses to
        raw = q_h · (W_key_h · kᵀ · G · [w_mu,w_sigma] / sqrt(D))        [Q,2]
    which is O(L·DM + DM) algebra — computed on HOST along with the
    sigmoid/softplus smalls, producing six per-query coefficient rows
        B6 = [rec_e, -2·mu·rec_e, mu²·rec_e+ln(2π s²_e),  (same for σ_o)]  [6,Q]
  * On device, r is produced by a rank-3 PE matmul + exp:
        y[n,q] = b_n²·B6₀ + b_n·B6₁ + B6₂   (per σ-group);   r = exp(-y/2)
  * Value path contracts k first:  kv = kᵀ-contract W_valᵀ, values = Gᵀ·kv.
  * Final projection: per-core partial over the core's 256 feature columns;
    host sums the 8 partials (no device collectives).

All PE matmuls use float32r moving operands with free-dim ≥ 256 (1 cycle/row
vs 4 for plain f32).
"""

import math
import numpy as np

import concourse.bass as bass
import concourse.mybir as mybir
import concourse.tile as tile
from concourse import bacc

F32 = mybir.dt.float32
BF16 = mybir.dt.bfloat16
AF = mybir.ActivationFunctionType
F32R = mybir.dt.float32r

H, D, N, L, Q = 16, 128, 1024, 512, 2048
DM = H * D            # 2048
NCORES = 8
HPC = H // NCORES     # heads per core = 2
DDC = HPC * D         # feature columns per core = 256
LN2PI = float(np.log(2.0 * np.pi))
OUT_BF16 = False
MM_DT = F32           # dtype for matmul operand tiles (F32R / F32 / BF16)

_G_CACHE = None


def _compute_G():
    """G = [l, N] ridge-regression basis projector; pure function of constants.

    Mirrors reference._compute_G (f32, jax on CPU) exactly.
    """
    global _G_CACHE
    if _G_CACHE is not None:
        return _G_CACHE
    import jax
    import jax.numpy as jnp

    with jax.default_device(jax.devices("cpu")[0]):
        n = N
        sigmas = (0.005, 0.01)
        m = jnp.linspace(0.0, 1.0, n // len(sigmas)).astype(jnp.float32)
        b_mu = jnp.repeat(m, len(sigmas))
        b_sigma = jnp.tile(jnp.asarray(sigmas, jnp.float32), n // len(sigmas))
        l = L
        shift = 1.0 / (2 * l)
        pos = jnp.linspace(-0.5 + shift, 1.5 - shift, 2 * l).astype(jnp.float32)
        x = (pos[None, :] - b_mu[:, None]) / b_sigma[:, None]
        F = jnp.exp(-0.5 * x * x) / (b_sigma[:, None] * jnp.sqrt(2.0 * jnp.pi))
        G = jnp.linalg.solve(F @ F.T + 0.5 * jnp.eye(n, dtype=jnp.float32), F).T
        G = G[l // 2 : -(l // 2)]
        _G_CACHE = np.asarray(G, dtype=np.float32)
    return _G_CACHE


def _build_bass(repeat=1):
    nc = bacc.Bacc("TRN2", target_bir_lowering=False)
    odt = BF16 if OUT_BF16 else F32

    # ---- DRAM I/O ----
    kT_d = nc.dram_tensor("kT", [DM, L], MM_DT, kind="ExternalInput")
    G_d = nc.dram_tensor("G", [L, N], MM_DT, kind="ExternalInput")
    lh6_d = nc.dram_tensor("lh6", [6, N], MM_DT, kind="ExternalInput")
    B6_d = nc.dram_tensor("B6", [HPC, 6, Q], MM_DT, kind="ExternalInput")
    WvT_d = nc.dram_tensor("WvT", [DM, DDC], MM_DT, kind="ExternalInput")
    WoT_d = nc.dram_tensor("WoT", [DDC, DM], MM_DT, kind="ExternalInput")
    out_d = nc.dram_tensor("out", [Q, DM], odt, kind="ExternalOutput")

    with tile.TileContext(nc) as tc:
        with (
            tc.tile_pool(name="singles", bufs=1) as singles,
            tc.tile_pool(name="rt", bufs=3) as rtp,
            tc.tile_pool(name="outp", bufs=2) as outp,
            tc.tile_pool(name="ps_d", bufs=2, space="PSUM") as ps_d,
            tc.tile_pool(name="ps_y", bufs=2, space="PSUM") as ps_y,
            tc.tile_pool(name="ps_c", bufs=2, space="PSUM") as ps_c,
            tc.tile_pool(name="ps_f", bufs=2, space="PSUM") as ps_f,
        ):
            for rep in range(repeat):
                # ---- persistent SBUF tensors (re-DMA'd each iteration) ----
                kT_sb = singles.tile([128, 16, L], MM_DT, tag="kT")
                for g in range(4):
                    nc.sync.dma_start(
                        out=kT_sb[:, 4 * g : 4 * (g + 1), :],
                        in_=kT_d[512 * g : 512 * (g + 1), :].rearrange(
                            "(t p) l -> p t l", p=128))
                WvT_sb = singles.tile([128, 16, DDC], MM_DT, tag="WvT")
                for g in range(4):
                    nc.gpsimd.dma_start(
                        out=WvT_sb[:, 4 * g : 4 * (g + 1), :],
                        in_=WvT_d[512 * g : 512 * (g + 1), :].rearrange(
                            "(t p) m -> p t m", p=128))
                G_sb = singles.tile([128, 4, N], MM_DT, tag="G")
                for g in range(2):
                    nc.sync.dma_start(
                        out=G_sb[:, 2 * g : 2 * (g + 1), :],
                        in_=G_d[256 * g : 256 * (g + 1), :].rearrange(
                            "(t p) n -> p t n", p=128))
                WoT_sb = singles.tile([128, HPC, DM], MM_DT, tag="WoT")
                nc.gpsimd.dma_start(
                    out=WoT_sb, in_=WoT_d[:].rearrange("(t p) j -> p t j", p=128))
                lh6_sb = singles.tile([6, N], MM_DT, tag="lh6")
                nc.sync.dma_start(out=lh6_sb, in_=lh6_d[:])
                B6_sb = singles.tile([6, HPC, Q], MM_DT, tag="B6")
                nc.sync.dma_start(out=B6_sb, in_=B6_d[:].rearrange("h p q -> p h q"))

                kv_sb = singles.tile([128, 4, DDC], MM_DT, tag="kv")      # [l%128, lt, m]
                values_sb = singles.tile([128, 8, DDC], MM_DT, tag="val")  # [n%128, nt, m]
                ctxT_sb = singles.tile([128, HPC, Q], MM_DT, tag="ctxT")   # [d%128, h, q]

                # ---- kv[l, m] = sum_c kT[c,l]·WvT[c,m]  (heads fused, m=256) ----
                for lt in range(4):
                    kv_ps = ps_d.tile([128, DDC], F32, tag="d_ps")
                    for ct in range(16):
                        nc.tensor.matmul(kv_ps,
                                         kT_sb[:, ct, lt * 128:(lt + 1) * 128],
                                         WvT_sb[:, ct, :],
                                         start=(ct == 0), stop=(ct == 15))
                    nc.vector.tensor_copy(out=kv_sb[:, lt, :], in_=kv_ps)

                # ---- values[n, m] = sum_l G[l,n]·kv[l,m] ----
                for nt in range(8):
                    v_ps = ps_d.tile([128, DDC], F32, tag="d_ps")
                    for lt in range(4):
                        nc.tensor.matmul(v_ps,
                                         G_sb[:, lt, nt * 128:(nt + 1) * 128],
                                         kv_sb[:, lt, :],
                                         start=(lt == 0), stop=(lt == 3))
                    nc.vector.tensor_copy(out=values_sb[:, nt, :], in_=v_ps)

                # ---- r-weighted context per head / q-chunk; then out projection ----
                for hl in range(HPC):
                    for ch in range(4):
                        c_ps = ps_c.tile([128, 512], F32, tag="c_ps")
                        for nt in range(8):
                            y_ps = ps_y.tile([128, 512], F32, tag="y_ps")
                            nc.tensor.matmul(y_ps,
                                             lh6_sb[:, nt * 128:(nt + 1) * 128],
                                             B6_sb[:, hl, ch * 512:(ch + 1) * 512],
                                             start=True, stop=True)
                            rT = rtp.tile([128, 512], MM_DT, tag="rT")
                            nc.scalar.activation(out=rT, in_=y_ps, func=AF.Exp,
                                                 scale=-0.5)
                            nc.tensor.matmul(c_ps,
                                             values_sb[:, nt, hl * 128:(hl + 1) * 128],
                                             rT,
                                             start=(nt == 0), stop=(nt == 7))
                        nc.vector.tensor_copy(
                            out=ctxT_sb[:, hl, ch * 512:(ch + 1) * 512], in_=c_ps)

                        # out[q, j] = sum_m ctx[q,m]·WoT[m,j], m = 2 heads x 128
                        if hl == HPC - 1:
                            for qt in range(4 * ch, 4 * ch + 4):
                                o_sb = outp.tile([128, DM], odt, tag="o_sb")
                                for jc in range(4):
                                    f_ps = ps_f.tile([128, 512], F32, tag="f_ps")
                                    for h2 in range(HPC):
                                        nc.tensor.matmul(
                                            f_ps,
                                            ctxT_sb[:, h2, qt * 128:(qt + 1) * 128],
                                            WoT_sb[:, h2, jc * 512:(jc + 1) * 512],
                                            start=(h2 == 0), stop=(h2 == HPC - 1))
                                    eng = (nc.vector.tensor_copy,
                                           nc.scalar.copy,
                                           nc.vector.tensor_copy,
                                           nc.scalar.copy)[jc]
                                    eng(out=o_sb[:, jc * 512:(jc + 1) * 512], in_=f_ps)
                                nc.sync.dma_start(
                                    out=out_d[qt * 128:(qt + 1) * 128, :], in_=o_sb)

    nc.compile()
    return nc


_NC_CACHE = None
_EXEC_CACHE = None


def _get_exec():
    """Build + cache the sharded jitted executable (compile once per process)."""
    global _NC_CACHE, _EXEC_CACHE
    if _EXEC_CACHE is not None:
        return _EXEC_CACHE
    if _NC_CACHE is None:
        _NC_CACHE = _build_bass()
    _EXEC_CACHE = _make_exec(_NC_CACHE)
    return _EXEC_CACHE


def _make_exec(nc):
    import jax
    from jax.experimental.shard_map import shard_map
    from jax.sharding import Mesh, PartitionSpec
    from concourse import bass2jax as b2j
    import concourse.mybir as _mybir

    b2j.install_neuronx_cc_hook()

    partition_name = nc.partition_id_tensor.name if nc.partition_id_tensor else None
    in_names, out_names, out_avals, zero_outs = [], [], [], []
    for alloc in nc.m.functions[0].allocations:
        if not isinstance(alloc, _mybir.MemoryLocationSet):
            continue
        name = alloc.memorylocations[0].name
        if alloc.kind == "ExternalInput":
            if name != partition_name:
                in_names.append(name)
        elif alloc.kind == "ExternalOutput":
            out_names.append(name)
            shape = tuple(alloc.tensor_shape)
            dtype = _mybir.dt.np(alloc.dtype)
            out_avals.append(jax.core.ShapedArray(shape, dtype))
            zero_outs.append(np.zeros(shape, dtype))
    n_params = len(in_names)
    n_outs = len(out_avals)
    all_in_names = in_names + out_names
    if partition_name is not None:
        all_in_names = all_in_names + [partition_name]

    def _body(*args):
        operands = list(args)
        if partition_name is not None:
            operands.append(b2j.partition_id_tensor())
        outs = b2j._bass_exec_p.bind(
            *operands,
            out_avals=tuple(out_avals),
            in_names=tuple(all_in_names),
            out_names=tuple(out_names),
            lowering_input_output_aliases=(),
            sim_require_finite=True,
            sim_require_nnan=True,
            nc=nc,
        )
        return tuple(outs)

    devices = jax.devices()[:NCORES]
    mesh = Mesh(np.asarray(devices), ("core",))
    sharded = jax.jit(
        shard_map(
            _body, mesh=mesh,
            in_specs=(PartitionSpec("core"),) * (n_params + n_outs),
            out_specs=(PartitionSpec("core"),) * n_outs,
            check_rep=False,
        ),
        donate_argnums=tuple(range(n_params, n_params + n_outs)),
        keep_unused=True,
    )
    return (sharded, in_names, out_names, out_avals, zero_outs)


def _softplus(x):
    return np.logaddexp(0.0, x)


def _prep_in_maps(k, q, W_key, W_val, W_out, w_mu, w_sigma):
    k = np.asarray(k, np.float32).reshape(L, DM)
    q = np.asarray(q, np.float32).reshape(H, Q, D)
    W_key = np.asarray(W_key, np.float32)
    W_val = np.asarray(W_val, np.float32)
    W_out = np.asarray(W_out, np.float32)
    w_mu = np.asarray(w_mu, np.float32)
    w_sigma = np.asarray(w_sigma, np.float32)

    G = _compute_G()                      # [L, N] f32
    # permutation: even basis indices (sigma=0.005) first
    perm = np.concatenate([np.arange(0, N, 2), np.arange(1, N, 2)])
    Gp = np.ascontiguousarray(G[:, perm])
    b_mu = np.repeat(np.linspace(0.0, 1.0, N // 2, dtype=np.float32), 2)[perm]

    # ---- host: mu/sigma functional path (O(L·DM + H·Q·D) algebra) ----
    wms = np.stack([w_mu, w_sigma], axis=1) / math.sqrt(D)         # [N, 2]
    gms = G @ wms                                                   # [L, 2]
    bms = k.T @ gms                                                 # [DM, 2]
    kms = (W_key @ bms).reshape(H, D, 2)                            # [H, D, 2]
    raw = np.einsum("hqd,hdw->hqw", q, kms, optimize=True)          # [H, Q, 2]
    mu = 1.0 / (1.0 + np.exp(-raw[..., 0].astype(np.float64)))
    s2 = np.maximum(_softplus(raw[..., 1].astype(np.float64)), 1e-4)
    B6 = np.empty((H, 6, Q), np.float32)
    for g, sg in enumerate((0.005, 0.01)):
        st = s2 + sg * sg
        rec = 1.0 / st
        B6[:, 3 * g + 0] = rec
        B6[:, 3 * g + 1] = -2.0 * mu * rec
        B6[:, 3 * g + 2] = mu * mu * rec + np.log(2.0 * np.pi * st)

    lh6 = np.zeros((6, N), np.float32)
    for t in range(8):
        sl = slice(t * 128, (t + 1) * 128)
        base = 0 if t < 4 else 3
        lh6[base + 0, sl] = b_mu[sl] ** 2
        lh6[base + 1, sl] = b_mu[sl]
        lh6[base + 2, sl] = 1.0

    kT = np.ascontiguousarray(k.T)                                  # [DM, L] f32

    in_maps = []
    for i in range(NCORES):
        hsl = slice(2 * i * D, (2 * i + 2) * D)
        WvT_loc = np.ascontiguousarray(W_val[hsl, :].T)             # [DM, 256] f32
        WoT_loc = np.ascontiguousarray(W_out[:, hsl].T)             # [256, DM] f32
        in_maps.append({
            "kT": kT, "G": Gp, "lh6": lh6,
            "B6": np.ascontiguousarray(B6[2 * i:2 * i + 2]),
            "WvT": WvT_loc, "WoT": WoT_loc,
        })
    return in_maps


def _concat_args(in_maps, exec_tuple=None):
    sharded, in_names, out_names, out_avals, zero_outs = exec_tuple or _get_exec()
    concat_in = [
        np.concatenate([np.asarray(in_maps[c][name]) for c in range(NCORES)], axis=0)
        for name in in_names
    ]
    concat_zeros = [
        np.zeros((NCORES * z.shape[0], *z.shape[1:]), z.dtype) for z in zero_outs
    ]
    return concat_in, concat_zeros


def kernel(k, q, W_key, W_val, W_out, w_mu, w_sigma, new_doc=None, **_unused):
    import jax
    k = np.asarray(k, np.float32).reshape(L, DM)
    q = np.asarray(q, np.float32).reshape(H, Q, D)
    in_maps = _prep_in_maps(k, q,
                            np.asarray(W_key, np.float32), np.asarray(W_val, np.float32),
                            np.asarray(W_out, np.float32),
                            np.asarray(w_mu, np.float32), np.asarray(w_sigma, np.float32))
    sharded, in_names, out_names, out_avals, zero_outs = _get_exec()
    concat_in, concat_zeros = _concat_args(in_maps)
    out_arrs = sharded(*concat_in, *concat_zeros)
    oi = out_names.index("out")
    parts = np.asarray(out_arrs[oi]).reshape(NCORES, Q, DM)
    out = parts.astype(np.float64).sum(axis=0)
    return out.astype(np.float32).reshape(1, Q, DM)
